# revision 4
# baseline (speedup 1.0000x reference)
"""Conv2d(128->256, 3x3, stride 1, pad 1) on (32,128,56,56) fp32, data-parallel over 8 NeuronCores.

Strategy per core (4 images):
  - Host pads x spatially to 58x58 so every conv window read is a clean strided
    SBUF access; host pre-transposes weight to [I=128, 9, O=256] so all DMAs are
    contiguous per partition.
  - Conv = 9 accumulating matmuls per output tile: out[o, h*56+w] += sum_i
    w[i, kh*3+kw, o] * xpad[i, (h+kh)*58 + (w+kw)].  K = I = 128 (partition dim),
    M = 128 (half of O=256), N = 448 (8 output rows x 56 cols, one PSUM bank).
  - Inputs are bitcast to float32r: PE reads fp32, truncates to FP22, runs at
    1 cycle/row for N>=256 (4x faster than true fp32; ~1e-4 rel err here).
  - PSUM -> SBUF copy fuses the bias add (ScalarE/VectorE alternating), then
    contiguous DMA to DRAM.
"""

import numpy as np

import concourse.bass as bass  # noqa: F401  (AP types come through bacc)
import concourse.mybir as mybir
import concourse.tile as tile
from concourse import bacc
from concourse.bass_utils import run_bass_kernel_spmd

N_CORES = 8
N_IMG = 4  # images per core
C_IN = 128
C_OUT = 256
H = W = 56
HP = WP = 58
SP = HP * WP  # 3364 padded spatial
SO = H * W  # 3136 output spatial
NROW = 8  # output rows per PSUM chunk
NCH = NROW * W  # 448 columns per matmul
RCHUNKS = H // NROW  # 7

_CACHE = {}


def _build_module():
    nc = bacc.Bacc("TRN2", target_bir_lowering=False, debug=False)

    f32 = mybir.dt.float32
    f32r = mybir.dt.float32r

    xp = nc.dram_tensor("xp", [N_IMG, C_IN, SP], f32r, kind="ExternalInput").ap()
    wt = nc.dram_tensor("wt", [C_IN, 9 * C_OUT], f32r, kind="ExternalInput").ap()
    br = nc.dram_tensor("br", [C_IN, 2], f32, kind="ExternalInput").ap()
    out = nc.dram_tensor("out", [N_IMG, C_OUT, SO], f32, kind="ExternalOutput").ap()

    with tile.TileContext(nc) as tc:
        with (
            tc.tile_pool(name="const", bufs=1) as cpool,
            tc.tile_pool(name="xin", bufs=1) as xpool,
            tc.tile_pool(name="osb", bufs=3) as opool,
            tc.tile_pool(name="pp", bufs=4, space="PSUM") as ppool,
        ):
            w_sb = cpool.tile([C_IN, 9, C_OUT], f32r)
            b_sb = cpool.tile([C_IN, 2], f32)
            nc.sync.dma_start(out=w_sb, in_=wt.rearrange("i (k o) -> i k o", k=9))
            nc.sync.dma_start(out=b_sb, in_=br)

            x_sb = xpool.tile([C_IN, N_IMG, SP], f32r)
            for n in range(N_IMG):
                nc.sync.dma_start(out=x_sb[:, n], in_=xp[n])
            # [C_IN, N_IMG, HP, WP] view for conv-window slicing
            x_v = x_sb.rearrange("c n (h w) -> c n h w", h=HP)

            for n in range(N_IMG):
                for o2 in range(2):
                    o_sb = opool.tile([128, SO], f32, tag="o_sb")
                    for r in range(RCHUNKS):
                        ps = ppool.tile([128, NCH], f32, tag="ps")
                        lhs_base = w_sb[:, :, o2 * 128 : (o2 + 1) * 128]
                        for kh in range(3):
                            for kw in range(3):
                                k = kh * 3 + kw
                                rhs = x_v[:, n, r * NROW + kh : r * NROW + kh + NROW, kw : kw + W]
                                nc.tensor.matmul(
                                    ps,
                                    lhsT=lhs_base[:, k],
                                    rhs=rhs,
                                    start=(k == 0),
                                    stop=(k == 8),
                                )
                        dst = o_sb[:, r * NCH : (r + 1) * NCH]
                        bias_ap = b_sb[:, o2 : o2 + 1]
                        if r % 2 == 0:
                            nc.vector.tensor_scalar_add(dst, ps, bias_ap)
                        else:
                            nc.scalar.activation(
                                dst, ps, mybir.ActivationFunctionType.Identity, bias=bias_ap
                            )
                        nc.sync.dma_start(
                            out=out[n, o2 * 128 : (o2 + 1) * 128, r * NCH : (r + 1) * NCH],
                            in_=dst,
                        )

    nc.compile()
    return nc


def _get_module():
    if "nc" not in _CACHE:
        _CACHE["nc"] = _build_module()
    return _CACHE["nc"]


def kernel(x, weight, bias):
    x = np.asarray(x, dtype=np.float32)
    weight = np.asarray(weight, dtype=np.float32)
    bias = np.asarray(bias, dtype=np.float32)

    xp = np.pad(x, ((0, 0), (0, 0), (1, 1), (1, 1))).reshape(32, C_IN, SP)
    wt = np.ascontiguousarray(weight.transpose(1, 2, 3, 0)).reshape(C_IN, 9 * C_OUT)
    br = np.ascontiguousarray(bias.reshape(2, 128).T)

    nc = _get_module()
    in_maps = [
        {"xp": np.ascontiguousarray(xp[N_IMG * c : N_IMG * (c + 1)]), "wt": wt, "br": br}
        for c in range(N_CORES)
    ]
    res = run_bass_kernel_spmd(nc, in_maps, core_ids=list(range(N_CORES)))
    outs = [r["out"].reshape(N_IMG, C_OUT, H, W) for r in res.results]
    return np.concatenate(outs, axis=0)


# revision 16
# speedup vs baseline: 35817.0486x; 35817.0486x over previous
"""Conv2d(128->256, 3x3, stride 1, pad 1) on (32,128,56,56) fp32, data-parallel over 8 NeuronCores.

Strategy per core (4 images):
  - Host pads x spatially to 58x58 so every conv window read is a clean strided
    SBUF access; host pre-transposes weight to [I=128, 9, O=256] so all DMAs are
    contiguous per partition.
  - Conv = 9 accumulating matmuls per output tile: out[o, h*56+w] += sum_i
    w[i, kh*3+kw, o] * xpad[i, (h+kh)*58 + (w+kw)].  K = I = 128 (partition dim),
    M = 128 (half of O=256), N = 448 (8 output rows x 56 cols, one PSUM bank).
  - Inputs are bitcast to float32r: PE reads fp32, truncates to FP22, runs at
    1 cycle/row for N>=256 (4x faster than true fp32; ~1e-4 rel err here).
  - PSUM -> SBUF copy fuses the bias add (ScalarE/VectorE alternating), then
    contiguous DMA to DRAM.
"""

import numpy as np

import concourse.bass as bass  # noqa: F401  (AP types come through bacc)
import concourse.mybir as mybir
import concourse.tile as tile
from concourse import bacc
from concourse.bass_utils import run_bass_kernel_spmd

N_CORES = 8
N_IMG = 4  # images per core
C_IN = 128
C_OUT = 256
H = W = 56
HP = WP = 58
SP = HP * WP  # 3364 padded spatial
SO = H * W  # 3136 output spatial
NROW = 8  # output rows per PSUM chunk
NCH = NROW * W  # 448 columns per matmul
RCHUNKS = H // NROW  # 7

_CACHE = {}


def _build_module(rchunks=RCHUNKS, repeat=1):
    nc = bacc.Bacc("TRN2", target_bir_lowering=False, debug=False)

    f32 = mybir.dt.float32
    f32r = mybir.dt.float32r

    xp = nc.dram_tensor("xp", [N_IMG, C_IN, SP], f32r, kind="ExternalInput").ap()
    wt = nc.dram_tensor("wt", [C_IN, 9 * C_OUT], f32r, kind="ExternalInput").ap()
    br = nc.dram_tensor("br", [C_IN, 2], f32, kind="ExternalInput").ap()
    out = nc.dram_tensor("out", [N_IMG, C_OUT, SO], f32, kind="ExternalOutput").ap()

    with tile.TileContext(nc) as tc:
        with (
            tc.tile_pool(name="const", bufs=1) as cpool,
            tc.tile_pool(name="xin", bufs=2 if repeat > 1 else 1) as xpool,
            tc.tile_pool(name="osb", bufs=3) as opool,
            tc.tile_pool(name="pp", bufs=6, space="PSUM") as ppool,
        ):
            w_sb = cpool.tile([C_IN, 9, C_OUT], f32r)
            b_sb = cpool.tile([C_IN, 2], f32)
            wt_v = wt.rearrange("i (k o) -> i k o", k=9)

            H_BANDS = [(0, 16), (16, 30), (30, 44), (44, HP)]

            def load_image(x_sb, n, first=False):
                if first:
                    # Head-critical pieces go FIRST on the SP sequencer so
                    # they are first in line at the shared DMA engines: rows
                    # 0-9 of image 0, then the o2=0 weight taps in
                    # consumption order. Everything else follows on gpsimd.
                    nc.sync.dma_start(out=x_sb[:, n, : 10 * WP], in_=xp[n, :, : 10 * WP])
                    for k0 in (0, 3, 6):
                        nc.sync.dma_start(
                            out=w_sb[:, k0 : k0 + 3, 0:128], in_=wt_v[:, k0 : k0 + 3, 0:128]
                        )
                    nc.gpsimd.dma_start(out=b_sb, in_=br)
                    nc.gpsimd.dma_start(
                        out=x_sb[:, n, 10 * WP : 24 * WP], in_=xp[n, :, 10 * WP : 24 * WP]
                    )
                    nc.gpsimd.dma_start(
                        out=x_sb[:, n, 24 * WP : 40 * WP], in_=xp[n, :, 24 * WP : 40 * WP]
                    )
                    nc.gpsimd.dma_start(out=x_sb[:, n, 40 * WP :], in_=xp[n, :, 40 * WP :])
                    nc.gpsimd.dma_start(out=w_sb[:, :, 128:256], in_=wt_v[:, :, 128:256])
                else:
                    for h0, h1 in H_BANDS:
                        nc.sync.dma_start(
                            out=x_sb[:, n, h0 * WP : h1 * WP],
                            in_=xp[n, :, h0 * WP : h1 * WP],
                        )

            first_rep = True
            for _rep in range(repeat):
                x_sb = xpool.tile([C_IN, N_IMG, SP], f32r, tag="x_sb")
                load_image(x_sb, 0, first=first_rep)
                first_rep = False
                # [C_IN, N_IMG, HP, WP] view for conv-window slicing
                x_v = x_sb.rearrange("c n (h w) -> c n h w", h=HP)

                for n in range(N_IMG):
                    if n + 1 < N_IMG:
                        # prefetch the next image one compute-block ahead
                        load_image(x_sb, n + 1)
                    for o2 in range(2):
                        o_sb = opool.tile([128, SO], f32, tag="o_sb")
                        for r in range(rchunks):
                            ps = ppool.tile([128, NCH], f32, tag="ps")
                            lhs_base = w_sb[:, :, o2 * 128 : (o2 + 1) * 128]
                            for kh in range(3):
                                for kw in range(3):
                                    k = kh * 3 + kw
                                    rhs = x_v[:, n, r * NROW + kh : r * NROW + kh + NROW, kw : kw + W]
                                    nc.tensor.matmul(
                                        ps,
                                        lhsT=lhs_base[:, k],
                                        rhs=rhs,
                                        start=(k == 0),
                                        stop=(k == 8),
                                    )
                            dst = o_sb[:, r * NCH : (r + 1) * NCH]
                            bias_ap = b_sb[:, o2 : o2 + 1]
                            if r % 2 == 0:
                                nc.vector.tensor_scalar_add(dst, ps, bias_ap)
                            else:
                                nc.scalar.activation(
                                    dst, ps, mybir.ActivationFunctionType.Identity, bias=bias_ap
                                )
                            nc.sync.dma_start(
                                out=out[n, o2 * 128 : (o2 + 1) * 128, r * NCH : (r + 1) * NCH],
                                in_=dst,
                            )

    nc.compile()
    return nc


def _get_module():
    if "nc" not in _CACHE:
        _CACHE["nc"] = _build_module()
    return _CACHE["nc"]


def kernel(x, weight, bias):
    x = np.asarray(x, dtype=np.float32)
    weight = np.asarray(weight, dtype=np.float32)
    bias = np.asarray(bias, dtype=np.float32)

    xp = np.pad(x, ((0, 0), (0, 0), (1, 1), (1, 1))).reshape(32, C_IN, SP)
    wt = np.ascontiguousarray(weight.transpose(1, 2, 3, 0)).reshape(C_IN, 9 * C_OUT)
    br = np.ascontiguousarray(bias.reshape(2, 128).T)

    nc = _get_module()
    in_maps = [
        {"xp": np.ascontiguousarray(xp[N_IMG * c : N_IMG * (c + 1)]), "wt": wt, "br": br}
        for c in range(N_CORES)
    ]
    res = run_bass_kernel_spmd(nc, in_maps, core_ids=list(range(N_CORES)))
    outs = [r["out"].reshape(N_IMG, C_OUT, H, W) for r in res.results]
    return np.concatenate(outs, axis=0)
